# revision 1
# baseline (speedup 1.0000x reference)
"""Trainium2 Bass kernel for nn_Attention_14955076125505.

Windowed self-attention with relative-position bias:
  x:(8,512,32,32) -> qkv -> 16-head attention(N=1024, d=32) + bias_table[rel_index]
  -> out proj -> (8,512,32,32)

Sharding (8 NeuronCores):
  - tensor-parallel over heads: core c owns heads (2c, 2c+1) for qkv + attention
  - bias gather is position-split: core c gathers keys [128c,128c+128) x all
    queries x all 16 heads via GpSimd ap_gather, exp()s them, then an AllToAll
    redistributes so core c holds exp(bias) for its 2 heads x all positions
  - after attention a second (tiny) AllToAll re-shards by batch so each core
    runs the full output projection for one batch with no reduction
Compute dtype bf16 on the TensorEngine (f32 PSUM accumulation), exp on ScalarE,
bias multiply on VectorE, softmax denominator via a ones-column in V.
"""

import sys

if "/opt/trn_rl_repo" not in sys.path:
    sys.path.insert(0, "/opt/trn_rl_repo")

import numpy as np
import ml_dtypes

B = 8
C = 512
N = 1024  # H*W
HEADS = 16
D = 32
OUP = 512
TABLE = 3969
NCORES = 8
HPC = 2  # heads per core
KCH = 128  # keys gathered per core
SCALE = D ** -0.5

BF = ml_dtypes.bfloat16

_GRAPH_CACHE = {}

E_SEL = np.zeros((64, 64), np.float32)
E_SEL[0, 0:32] = 1.0
E_SEL[32, 32:64] = 1.0


def _build_graph(repeat=1, collectives=True, num_devices=NCORES, skip_bias=False, skip_exp=False, il_scores=True, bias_q='sync', qkv_split=False, at_bufs=8, big_st=True, skip_mult=False, fast_scatter=True):
    import concourse.bass as bass
    import concourse.mybir as mybir
    import concourse.tile as tile
    from concourse import bacc

    fp32 = mybir.dt.float32
    bf16 = mybir.dt.bfloat16
    i16 = mybir.dt.int16

    nc = bacc.Bacc(
        "TRN2",
        target_bir_lowering=False,
        debug=False,
        enable_asserts=True,
        num_devices=num_devices,
    )

    # ---- kernel I/O (per-core shards, prepared host-side) ----
    x_d = nc.dram_tensor("x", [B, C, N], bf16, kind="ExternalInput").ap()
    wqk_d = nc.dram_tensor("w_qkT", [C, 128], bf16, kind="ExternalInput").ap()
    wv_d = nc.dram_tensor("w_vT", [C, 2 * D], bf16, kind="ExternalInput").ap()
    wo_d = nc.dram_tensor("w_outT", [OUP, OUP], bf16, kind="ExternalInput").ap()
    bo_d = nc.dram_tensor("b_out4", [128, 4], fp32, kind="ExternalInput").ap()
    tab_d = nc.dram_tensor("table", [128, TABLE], fp32, kind="ExternalInput").ap()
    idx_d = nc.dram_tensor("idx", [128, N], i16, kind="ExternalInput").ap()
    esel_d = nc.dram_tensor("e_sel", [64, 64], fp32, kind="ExternalInput").ap()
    out_d = nc.dram_tensor("out", [OUP, N], fp32, kind="ExternalOutput").ap()

    # ---- internal DRAM bounce buffers for the collectives ----
    POS = KCH * N  # positions gathered per core = 131072
    eb_in = nc.dram_tensor("eb_a2a_in", [HPC, NCORES, POS], bf16).ap()
    ebc_in = nc.dram_tensor("ebc_a2a_in", [16, 8, 4, 4096], bf16).ap()  # [r, g, chunk, f] (dest-major)
    ebc_out = nc.dram_tensor("ebc_a2a_out", [NCORES, HPC, 8, 4, 4096], bf16).ap()
    eb_out = nc.dram_tensor("eb_a2a_out", [HPC, NCORES, POS], bf16).ap()
    ao_in = nc.dram_tensor("ao_a2a_in", [NCORES, 2 * D, N], bf16).ap()
    ao_out = nc.dram_tensor("ao_a2a_out", [NCORES, 2 * D, N], bf16).ap()

    RG = [list(range(NCORES))]

    with tile.TileContext(nc) as tc:
        _bq = {'gpsimd': nc.gpsimd, 'sync': nc.sync, 'scalar': nc.scalar}[bias_q]
        with (
            tc.tile_pool(name="const", bufs=1) as cp,
            tc.tile_pool(name="persist", bufs=1) as pp,
            tc.tile_pool(name="work", bufs=3) as wp,
            tc.tile_pool(name="small", bufs=1) as sp,
            tc.tile_pool(name="psum", bufs=2, space="PSUM") as psp,
            tc.tile_pool(name="psav", bufs=1, space="PSUM") as pav,
        ):
            # ================= constants =================
            tab_sb = cp.tile([128, TABLE], fp32)
            nc.sync.dma_start(tab_sb[:, :], tab_d)
            idx_sb = cp.tile([128, N], i16)
            nc.sync.dma_start(idx_sb[:, :], idx_d)
            wqk_sb = cp.tile([128, 4, 128], bf16)
            nc.sync.dma_start(wqk_sb[:, :, :], wqk_d.rearrange("(kc p) m -> p kc m", p=128))
            wv_sb = cp.tile([128, 4, 2 * D], bf16)
            nc.sync.dma_start(wv_sb[:, :, :], wv_d.rearrange("(kc p) m -> p kc m", p=128))
            wo_sb = cp.tile([128, 4, OUP], bf16)
            nc.sync.dma_start(wo_sb[:, :, :], wo_d.rearrange("(kc p) m -> p kc m", p=128))
            bo_sb = cp.tile([128, 4], fp32)
            nc.sync.dma_start(bo_sb[:, :], bo_d)

            for _rep in range(repeat):
                # ================= qkv projection (all 8 batches) =================
                # q_sb/k_sb: [h_loc*32+d, batch, token]  (d-major, heads at partition 0/32)
                q_sb = pp.tile([64, B, N], bf16)
                k_sb = pp.tile([64, B, N], bf16)
                # v_sb: [token_in_block, batch, tb, h_loc, 33] with ones in col 32
                v_sb = pp.tile([128, B, 8, HPC, D + 1], bf16)
                nc.vector.memset(v_sb[:, :, :, :, D], 1.0)

                for b in range(B):
                    x_t = wp.tile([128, 4, N], bf16, tag="xt")
                    nc.sync.dma_start(x_t[:, :, :], x_d[b].rearrange("(kc p) n -> p kc n", p=128))

                    if qkv_split:
                        q_ps = psp.tile([64, N], fp32, tag="st", bufs=(1 if big_st else 3), padded_shape=[128, N], name="q_ps")
                        k_ps = psp.tile([64, N], fp32, tag="st", bufs=(1 if big_st else 3), padded_shape=[128, N], name="k_ps")
                        for half in range(2):
                            sl = slice(512 * half, 512 * (half + 1))
                            for kc in range(4):
                                nc.tensor.matmul(
                                    q_ps[:, sl],
                                    wqk_sb[:, kc, 0:64],
                                    x_t[:, kc, sl],
                                    start=(kc == 0),
                                    stop=(kc == 3),
                                )
                            for kc in range(4):
                                nc.tensor.matmul(
                                    k_ps[:, sl],
                                    wqk_sb[:, kc, 64:128],
                                    x_t[:, kc, sl],
                                    start=(kc == 0),
                                    stop=(kc == 3),
                                )
                        nc.vector.tensor_copy(q_sb[:, b, :], q_ps[:, :])
                        nc.vector.tensor_copy(k_sb[:, b, :], k_ps[:, :])
                    else:
                        qk_ps = psp.tile([128, N], fp32, tag="st", bufs=(1 if big_st else 3))
                        for half in range(2):
                            sl = slice(512 * half, 512 * (half + 1))
                            for kc in range(4):
                                nc.tensor.matmul(
                                    qk_ps[:, sl],
                                    wqk_sb[:, kc, :],
                                    x_t[:, kc, sl],
                                    start=(kc == 0),
                                    stop=(kc == 3),
                                )
                        nc.vector.tensor_copy(q_sb[:, b, :], qk_ps[0:64, :])
                        nc.scalar.copy(k_sb[:, b, :], qk_ps[64:128, :])

                    for tb in range(8):
                        v_ps = psp.tile([128, 2 * D], fp32, tag="st", bufs=(1 if big_st else 3), padded_shape=[128, N])
                        for kc in range(4):
                            nc.tensor.matmul(
                                v_ps[:, :],
                                x_t[:, kc, 128 * tb:128 * (tb + 1)],
                                wv_sb[:, kc, :],
                                start=(kc == 0),
                                stop=(kc == 3),
                            )
                        nc.vector.tensor_copy(v_sb[:, b, tb, :, 0:D], v_ps[:, :])

                # ================= bias gather + exp + AllToAll =================
                if skip_bias:
                    eb_sb = pp.tile([128, 8, 2 * N], bf16, name="eb_sb")
                    nc.vector.memset(eb_sb[:, :, :], 1.0)
                else:
                    with tc.tile_pool(name="gather", bufs=1) as gp:

                        NIDX = 16 * N  # 16384 indices per 16-partition group
                        NCHUNK = 4
                        CH = NIDX // NCHUNK  # 4096 indices per ap_gather call
                        for ch in range(NCHUNK):
                            gath = gp.tile([128, CH], fp32, tag="gath")
                            nc.gpsimd.ap_gather(
                                out_ap=gath[:, :],
                                in_ap=tab_sb[:, :],
                                idxs_ap=idx_sb[:, ch * (CH // 16):(ch + 1) * (CH // 16)],
                                channels=128,
                                num_elems=TABLE,
                                d=1,
                                num_idxs=CH,
                            )
                            eb_raw = gp.tile([128, CH], bf16, tag="ebraw")
                            nc.scalar.activation(eb_raw[:, :], gath[:, :], mybir.ActivationFunctionType.Exp)

                            # scatter to the collective input: dest core j gets heads (2j, 2j+1)
                            if fast_scatter:
                                _bq.dma_start(
                                    ebc_in[:, :, ch, :].rearrange("r g f -> g r f"),
                                    eb_raw[:, :],
                                )
                            else:
                                ebr = eb_raw.rearrange("(g r) f -> g r f", r=16)
                                for j in range(NCORES):
                                    for e in range(HPC):
                                        _bq.dma_start(
                                            eb_in[e, j].rearrange("(g f) -> g f", g=8)[:, ch * CH:(ch + 1) * CH],
                                            ebr[:, HPC * j + e, :],
                                        )
                        if fast_scatter:
                            nc.gpsimd.collective_compute(
                                "AllToAll",
                                mybir.AluOpType.bypass,
                                replica_groups=RG,
                                ins=[ebc_in.opt()],
                                outs=[ebc_out.opt()],
                            )
                        else:
                            for e in range(HPC):
                                if collectives:
                                    nc.gpsimd.collective_compute(
                                        "AllToAll",
                                        mybir.AluOpType.bypass,
                                        replica_groups=RG,
                                        ins=[eb_in[e].opt()],
                                        outs=[eb_out[e].opt()],
                                    )
                                else:
                                    nc.sync.dma_start(eb_out[e].opt(), eb_in[e].opt())

                    # exp(bias^T) for my 2 heads: [key-in-block, kb, h*N + q]
                    eb_sb = pp.tile([128, 8, 2 * N], bf16)
                    for kb in range(8):
                        for e in range(HPC):
                            if fast_scatter:
                                _bq.dma_start(
                                    eb_sb[:, kb, e * N:(e + 1) * N],
                                    ebc_out[kb, e].rearrange("g ch (fq q) -> (g ch fq) q", q=N),
                                )
                            else:
                                _bq.dma_start(
                                    eb_sb[:, kb, e * N:(e + 1) * N],
                                    eb_out[e, kb].rearrange("(p f) -> p f", p=128),
                                )


                # ================= attention =================
                # attnout_sb: [h_loc*32+d, batch, q] bf16 (normalized attention output^T)
                attnout_sb = pp.tile([64, B, N], bf16)

                # selector for broadcasting per-head reciprocals across 32 partitions:
                # bc[32h+d, q] = (E.T @ rec)[.,q] with E[h, 32h:32h+32] = 1
                e_sel = cp.tile([64, 64], fp32)
                nc.sync.dma_start(e_sel[:, :], esel_d)
                rec = cp.tile([64, N], fp32)
                nc.vector.memset(rec[:, :], 1.0)

                for b in range(B):
                    av_ps = pav.tile([128, N], fp32, tag="av", bufs=1)
                    for kb in range(8):
                        if big_st:
                            st2 = psp.tile([128, 2 * N], fp32, tag="st2", bufs=1)
                            for half in range(2):
                                sl = slice(512 * half, 512 * (half + 1))
                                for h in range(HPC):
                                    hp = slice(32 * h, 32 * (h + 1))
                                    nc.tensor.matmul(
                                        st2[:, h * N + 512 * half:h * N + 512 * (half + 1)],
                                        k_sb[hp, b, 128 * kb:128 * (kb + 1)],
                                        q_sb[hp, b, sl],
                                        start=True,
                                        stop=True,
                                    )
                            at2 = wp.tile([128, 2 * N], bf16, tag="attn2", bufs=at_bufs // 2)
                            nc.scalar.activation(at2[:, :], st2[:, :], mybir.ActivationFunctionType.Exp)
                            if not skip_mult:
                                nc.vector.tensor_mul(at2[:, :], at2[:, :], eb_sb[:, kb, :])
                            for half in range(2):
                                sl = slice(512 * half, 512 * (half + 1))
                                for h in range(HPC):
                                    nc.tensor.matmul(
                                        av_ps[64 * h:64 * h + D + 1, sl],
                                        v_sb[:, b, kb, h, :],
                                        at2[:, h * N + 512 * half:h * N + 512 * (half + 1)],
                                        start=(kb == 0),
                                        stop=(kb == 7),
                                    )
                        elif il_scores:
                            sts = []
                            for h in range(HPC):
                                st = psp.tile([128, N], fp32, tag="st", bufs=(1 if big_st else 3), name=f"st{h}")
                                sts.append(st)
                            for half in range(2):
                                sl = slice(512 * half, 512 * (half + 1))
                                for h in range(HPC):
                                    hp = slice(32 * h, 32 * (h + 1))
                                    nc.tensor.matmul(
                                        sts[h][:, sl],
                                        k_sb[hp, b, 128 * kb:128 * (kb + 1)],
                                        q_sb[hp, b, sl],
                                        start=True,
                                        stop=True,
                                    )
                            ats = []
                            for h in range(HPC):
                                st = sts[h]
                                at = wp.tile([128, N], bf16, tag="attn", bufs=at_bufs, name=f"at{h}")
                                if skip_exp:
                                    nc.vector.tensor_copy(at[:, :], st[:, :])
                                else:
                                    nc.scalar.activation(at[:, :], st[:, :], mybir.ActivationFunctionType.Exp)
                                if not skip_mult:
                                    nc.vector.tensor_mul(at[:, :], at[:, :], eb_sb[:, kb, h * N:(h + 1) * N])
                                ats.append(at)
                            for half in range(2):
                                sl = slice(512 * half, 512 * (half + 1))
                                for h in range(HPC):
                                    nc.tensor.matmul(
                                        av_ps[64 * h:64 * h + D + 1, sl],
                                        v_sb[:, b, kb, h, :],
                                        ats[h][:, sl],
                                        start=(kb == 0),
                                        stop=(kb == 7),
                                    )
                        else:
                            for h in range(HPC):
                                hp = slice(32 * h, 32 * (h + 1))
                                st = psp.tile([128, N], fp32, tag="st", bufs=(1 if big_st else 3))
                                for half in range(2):
                                    sl = slice(512 * half, 512 * (half + 1))
                                    nc.tensor.matmul(
                                        st[:, sl],
                                        k_sb[hp, b, 128 * kb:128 * (kb + 1)],
                                        q_sb[hp, b, sl],
                                        start=True,
                                        stop=True,
                                    )
                                at = wp.tile([128, N], bf16, tag="attn", bufs=at_bufs)
                                if skip_exp:
                                    nc.vector.tensor_copy(at[:, :], st[:, :])
                                else:
                                    nc.scalar.activation(at[:, :], st[:, :], mybir.ActivationFunctionType.Exp)
                                nc.vector.tensor_mul(at[:, :], at[:, :], eb_sb[:, kb, h * N:(h + 1) * N])
                                for half in range(2):
                                    sl = slice(512 * half, 512 * (half + 1))
                                    nc.tensor.matmul(
                                        av_ps[64 * h:64 * h + D + 1, sl],
                                        v_sb[:, b, kb, h, :],
                                        at[:, sl],
                                        start=(kb == 0),
                                        stop=(kb == 7),
                                    )
                    # normalize by the ones-column sums (rows 32 and 96 of av_ps)
                    nc.vector.reciprocal(rec[0:1, :], av_ps[D:D + 1, :])
                    nc.vector.reciprocal(rec[32:33, :], av_ps[64 + D:64 + D + 1, :])
                    bc_ps = psp.tile([64, N], fp32, tag="st", bufs=(1 if big_st else 3), padded_shape=[128, N])
                    for half in range(2):
                        sl = slice(512 * half, 512 * (half + 1))
                        nc.tensor.matmul(bc_ps[:, sl], e_sel[:, :], rec[:, sl], start=True, stop=True)
                    bc_sb = sp.tile([64, N], fp32, tag="bcs")
                    nc.scalar.copy(bc_sb[:, :], bc_ps[:, :])
                    for h in range(HPC):
                        nc.vector.tensor_mul(
                            attnout_sb[32 * h:32 * (h + 1), b, :],
                            av_ps[64 * h:64 * h + D, :],
                            bc_sb[32 * h:32 * (h + 1), :],
                        )

                # ================= all-to-all: heads -> batch =================
                for j in range(NCORES):
                    nc.sync.dma_start(ao_in[j], attnout_sb[:, j, :])
                if collectives:
                    nc.gpsimd.collective_compute(
                        "AllToAll",
                        mybir.AluOpType.bypass,
                        replica_groups=RG,
                        ins=[ao_in.opt()],
                        outs=[ao_out.opt()],
                    )
                else:
                    nc.sync.dma_start(ao_out.opt(), ao_in.opt())

                # ================= output projection (my batch) =================
                ao_sb = pp.tile([128, 4, N], bf16)
                for kc in range(4):
                    nc.sync.dma_start(
                        ao_sb[:, kc, :],
                        ao_out[2 * kc:2 * kc + 2].rearrange("j p n -> (j p) n"),
                    )
                for mb in range(4):
                    o_ps = psp.tile([128, N], fp32, tag="st", bufs=(1 if big_st else 3))
                    for half in range(2):
                        sl = slice(512 * half, 512 * (half + 1))
                        for kc in range(4):
                            nc.tensor.matmul(
                                o_ps[:, sl],
                                wo_sb[:, kc, 128 * mb:128 * (mb + 1)],
                                ao_sb[:, kc, sl],
                                start=(kc == 0),
                                stop=(kc == 3),
                            )
                    o_sb = wp.tile([128, N], fp32, tag="osb")
                    nc.vector.tensor_scalar_add(o_sb[:, :], o_ps[:, :], bo_sb[:, mb:mb + 1])
                    nc.sync.dma_start(out_d[128 * mb:128 * (mb + 1), :], o_sb[:, :])

    nc.compile()
    return nc


def _prepare_in_maps(inputs):
    x = np.asarray(inputs["x"], np.float32).reshape(B, C, N)
    w_qkv = np.asarray(inputs["w_qkv"], np.float32)
    w_out = np.asarray(inputs["w_out"], np.float32)
    b_out = np.asarray(inputs["b_out"], np.float32)
    table = np.asarray(inputs["bias_table"], np.float32)
    ridx = np.asarray(inputs["rel_index"]).astype(np.int64).reshape(N, N)

    x_bf = np.ascontiguousarray(x.astype(BF))
    wq = w_qkv[0:OUP]
    wk = w_qkv[OUP:2 * OUP]
    wv = w_qkv[2 * OUP:3 * OUP]
    w_outT = np.ascontiguousarray(w_out.T.astype(BF))
    b_out4 = np.ascontiguousarray(b_out.reshape(4, 128).T.astype(np.float32))
    tab_rep = np.ascontiguousarray(table.T[np.arange(128) % HEADS].astype(np.float32))

    in_maps = []
    for c in range(NCORES):
        h0, h1 = 2 * c, 2 * c + 1
        wqk_cols = np.concatenate(
            [
                wq[h0 * D:(h0 + 1) * D] * SCALE,
                wq[h1 * D:(h1 + 1) * D] * SCALE,
                wk[h0 * D:(h0 + 1) * D],
                wk[h1 * D:(h1 + 1) * D],
            ],
            axis=0,
        )  # [128, C]
        w_qkT = np.ascontiguousarray(wqk_cols.T.astype(BF))
        wv_cols = np.concatenate(
            [wv[h0 * D:(h0 + 1) * D], wv[h1 * D:(h1 + 1) * D]], axis=0
        )  # [64, C]
        w_vT = np.ascontiguousarray(wv_cols.T.astype(BF))

        # gather indices: keys [128c, 128c+128) x all queries, bias^T order
        sl = ridx[:, KCH * c:KCH * (c + 1)]  # [q, k_in]
        idxw = np.empty((128, N), np.int16)
        for g in range(8):
            arr = sl[:, 16 * g:16 * (g + 1)].T.reshape(-1)  # i = k_loc*N + q
            idxw[16 * g:16 * (g + 1)] = arr.reshape(N, 16).T
        in_maps.append(
            {
                "x": x_bf,
                "w_qkT": w_qkT,
                "w_vT": w_vT,
                "w_outT": w_outT,
                "b_out4": b_out4,
                "table": tab_rep,
                "e_sel": E_SEL,
                "idx": np.ascontiguousarray(idxw),
            }
        )
    return in_maps


def _get_graph(repeat=1, collectives=True, num_devices=NCORES, skip_bias=False, skip_exp=False, il_scores=True, bias_q='sync', qkv_split=False, at_bufs=8, big_st=True, skip_mult=False, fast_scatter=True):
    key = (repeat, collectives, num_devices, skip_bias, skip_exp, il_scores, bias_q, qkv_split, at_bufs, big_st, skip_mult, fast_scatter)
    if key not in _GRAPH_CACHE:
        _GRAPH_CACHE[key] = _build_graph(repeat, collectives, num_devices, skip_bias, skip_exp, il_scores, bias_q, qkv_split, at_bufs, big_st, skip_mult, fast_scatter)
    return _GRAPH_CACHE[key]


def run_on_hw(inputs, trace=False, **kw):
    from concourse.bass_utils import run_bass_kernel_spmd

    nc = _get_graph()
    in_maps = _prepare_in_maps(inputs)
    return run_bass_kernel_spmd(nc, in_maps, core_ids=list(range(NCORES)), trace=trace, **kw)


def kernel(**inputs) -> np.ndarray:
    res = run_on_hw(inputs).results
    out = np.stack([np.asarray(res[c]["out"], np.float32) for c in range(NCORES)])
    return out.reshape(B, OUP, 32, 32)


if __name__ == "__main__":
    _get_graph()
    print("graph built + compiled OK")

